# revision 7
# baseline (speedup 1.0000x reference)
"""Trainium2 Bass kernel for nn_KnotEntangle (B=8, K=32, S=256, L=8), v4.

Math collapse (same as v2/v3): smearWindow has lower == upper, so xStep == 0
and sig is a DC spike; the [B,K,K,S] pairwise block folds to per-j scalars
and out[b, s] = g[b, s] * res[b].

v4 layout: the sigma chain runs TRANSPOSED [8l x 32k] so the outer product
sigma sigma^T = esmT^T @ esmT comes straight from one PE matmul - no
accumulator read, no StreamTranspose, two fewer cross-engine hops.  sigma's
column/row-broadcast views also come from single matmuls against esmT.
out8 runs on Pool (PSUM-pointer reads) so the scatter trigger is same-queue.
Output via prepared dma_scatter_add onto a pre-zeroed DRAM target.
"""

import math

import numpy as np

import concourse.bacc as bacc
import concourse.bass as bass
import concourse.mybir as mybir
import concourse.tile as tile
from concourse import bass_utils

B, K, S, L = 8, 32, 256, 8
W = 154
F32 = mybir.dt.float32
I16 = mybir.dt.int16
I32 = mybir.dt.int32
AF = mybir.ActivationFunctionType
ALU = mybir.AluOpType
SQ2S = float(S * math.sqrt(2.0))

# column map: cols 0:128 hold the transposed sigma blocks in rows 0:8
# (kmT 0:32 | dvST 32:64 | eahT 64:96 | xB8 96:128); per-k scalars are
# replicated so partition p holds the entry for knot k = p % 32.
C_KMT = 0
C_DVST = 32
C_EAHT = 64
C_XB8 = 96
C_X = 128
C_EMS = 129
C_DVM = 130
C_EHM = 131
C_QQ8 = 132    # 132:140
C_AW = 140
C_AB = 141
C_T34LK = 142
C_DLHK = 143
C_M2L = 144
C_M2H = 145
C_INDIC = 146  # 146:154

_NC_CACHE = {}


def _build_nc() -> bacc.Bacc:
    nc = bacc.Bacc("TRN2", target_bir_lowering=False, debug=False)
    cols_d = nc.dram_tensor("cols", [128, W], F32, kind="ExternalInput")
    out_d = nc.dram_tensor("out", [16, 64], F32, kind="ExternalOutput")

    with tile.TileContext(nc) as tc:
        with (
            tc.tile_pool(name="sb", bufs=1) as sb,
            tc.tile_pool(name="ps", bufs=8, space="PSUM") as ps,
        ):
            cols = sb.tile([128, W], F32)
            nc.sync.dma_start(cols[:], cols_d.ap()[:, :])

            z16 = sb.tile([16, 64], F32)
            nc.vector.memset(z16[:], 0.0)
            nc.sync.dma_start(out_d.ap()[:, :], z16[:])

            src = sb.tile([128, 64], F32)
            nc.vector.memset(src[:], 0.0)

            dummy = sb.tile([1, 1], F32)
            nc.vector.memset(dummy[:], 0.0)
            dummy2 = sb.tile([1, 1], F32)
            nc.scalar.activation(dummy2[:], dummy[:], AF.Exp, scale=1.0,
                                 bias=dummy[:])

            # ---- const tiles (during the DMA wait)
            ones8 = sb.tile([L, K], F32)         # sgB lhsT / sgcol rhs
            nc.vector.memset(ones8[:], 1.0)
            ones32 = sb.tile([K, 128], F32)      # b1 lhsT
            nc.vector.memset(ones32[:], 1.0)
            c8 = sb.tile([K, 8], F32)
            nc.vector.memset(c8[:], 0.0)
            nc.vector.memset(c8[:, 0:8:2], 1.0)
            evens8 = sb.tile([L, 8], F32)        # res-seed lhsT: 1 at even r
            nc.vector.memset(evens8[:], 0.0)
            nc.vector.memset(evens8[:, 0:8:2], 1.0)
            qcol = sb.tile([128, 1], F32)
            for g in range(4):
                nc.vector.memset(qcol[32 * g:32 * g + 32, :],
                                 (1.0 + 64.0 * g) / 256.0)

            idxI = sb.tile([128, 1], I16)
            idxs = sb.tile([128, 1], I16)
            dma_sem = tc.sems.swdge_block()[0]
            with tc.high_priority():
                nc.gpsimd.iota(idxI[:], [[0, 1]], channel_multiplier=1)
                nc.vector.tensor_scalar(idxs[:], idxI[:], 15, None,
                                        ALU.bitwise_and)
                nc.gpsimd.dma_scatter_add(
                    out_d.ap()[:, :], src[:].unsqueeze(1), idxs[:], 16, 16,
                    64, prepare_only=True, sem=dma_sem,
                )

            # eye / C*eye for the PSUM diagonal offset (kills j == i via the
            # exp clamp: dd-diag gets +BIGC, so Mx-diag underflows to 0)
            iotaP = sb.tile([K, 1], F32)
            nc.gpsimd.iota(iotaP[:], [[0, 1]], channel_multiplier=1,
                           allow_small_or_imprecise_dtypes=True)
            iotaF32 = sb.tile([K, K], F32)
            nc.gpsimd.iota(iotaF32[:], [[1, K]], channel_multiplier=0,
                           allow_small_or_imprecise_dtypes=True)
            dPF = sb.tile([K, K], F32)
            nc.gpsimd.tensor_scalar(dPF[:], iotaF32[:], iotaP[:], None,
                                    ALU.subtract)
            eGE = sb.tile([K, K], F32)
            nc.gpsimd.tensor_scalar(eGE[:], dPF[:], 0.0, None, ALU.is_ge)
            eLE = sb.tile([K, K], F32)
            nc.gpsimd.tensor_scalar(eLE[:], dPF[:], 0.0, None, ALU.is_le)
            eye32 = sb.tile([K, K], F32)
            nc.gpsimd.tensor_mul(eye32[:], eGE[:], eLE[:])
            Ceye = sb.tile([K, K], F32)
            nc.gpsimd.tensor_scalar(Ceye[:], eye32[:], 1.0e3, None, ALU.mult)

            iotaF64 = sb.tile([128, 64], I32)
            nc.gpsimd.iota(iotaF64[:], [[1, 64]], channel_multiplier=0)
            xitF = sb.tile([128, 64], F32)
            nc.gpsimd.tensor_scalar(xitF[:], iotaF64[:], 1.0 / 256, qcol[:],
                                    ALU.mult, ALU.add)

            # ---- views
            kmT = cols[0:L, C_KMT:C_KMT + K]
            dvST = cols[0:L, C_DVST:C_DVST + K]
            eahT = cols[0:L, C_EAHT:C_EAHT + K]
            xB8 = cols[0:L, C_XB8:C_XB8 + K]
            x32 = cols[0:K, C_X:C_X + 1]
            x128 = cols[:, C_X:C_X + 1]
            emS_c = cols[0:K, C_EMS:C_EMS + 1]
            dvm_c = cols[0:K, C_DVM:C_DVM + 1]
            ehm_c = cols[0:K, C_EHM:C_EHM + 1]
            qq8 = cols[0:K, C_QQ8:C_QQ8 + 8]
            aw_c = cols[:, C_AW:C_AW + 1]
            ab_c = cols[:, C_AB:C_AB + 1]
            t34lk_c = cols[:, C_T34LK:C_T34LK + 1]
            dlhk_c = cols[:, C_DLHK:C_DLHK + 1]
            m2l_c = cols[:, C_M2L:C_M2L + 1]
            m2h_c = cols[:, C_M2H:C_M2H + 1]
            indic = cols[:, C_INDIC:C_INDIC + 8]

            # ---- PSUM tiles
            b1 = ps.tile([128, 1], F32, tag="ps")
            dM = ps.tile([K, K], F32, tag="ps")
            sgB = ps.tile([K, K], F32, tag="ps")
            sgcol = ps.tile([K, 1], F32, tag="ps")
            res8 = ps.tile([8, 1], F32, tag="ps")
            g8 = ps.tile([8, 64], F32, tag="ps")

            # ================= post-data program =================
            nc.tensor.matmul(b1[:], ones32[:], x32, skip_group_check=True)

            # am_neg = -(x*aw + ab): aw/ab columns are host-negated so this
            # lands directly in ACT-Identity bias form for aLm
            am_neg = sb.tile([128, 1], F32)
            nc.gpsimd.tensor_scalar(am_neg[:], x128, aw_c, ab_c,
                                    ALU.mult, ALU.add)

            # DVE sigma chain, transposed [8, 32], all same-queue
            ndT = sb.tile([L, K], F32)
            nc.vector.tensor_sub(ndT[:], kmT, xB8)
            mdT = sb.tile([L, K], F32)
            nc.vector.scalar_tensor_tensor(mdT[:], ndT[:], 0.0, dvST,
                                           ALU.is_ge, ALU.mult)
            selT = sb.tile([L, K], F32)
            nc.vector.tensor_add(selT[:], mdT[:], eahT)
            d2T = sb.tile([L, K], F32)
            nc.vector.tensor_mul(d2T[:], ndT[:], ndT[:])
            z2T = sb.tile([L, K], F32)
            nc.vector.tensor_mul(z2T[:], selT[:], d2T[:])

            # ACT: gate stds off the PSUM accumulator (ready before esmT)
            eLg = sb.tile([128, 1], F32)
            nc.scalar.activation(eLg[:], b1[:], AF.Exp, scale=m2l_c, bias=0.0)
            eHg = sb.tile([128, 1], F32)
            nc.scalar.activation(eHg[:], b1[:], AF.Exp, scale=m2h_c, bias=0.0)

            # ACT: gate prefix off the PSUM accumulator (GPSIMD can't read
            # PSUM on real HW; ACT [128,1] ops are nearly free)
            diffc = sb.tile([128, 1], F32)
            nc.scalar.activation(diffc[:], b1[:], AF.Identity, scale=dlhk_c,
                                 bias=0.0)
            aLm = sb.tile([128, 1], F32)
            nc.scalar.activation(aLm[:], b1[:], AF.Identity, scale=t34lk_c,
                                 bias=am_neg[:])

            # ACT: esmT = exp(-z2T/2)  (no accumulate needed)
            esmT = sb.tile([L, K], F32)
            nc.scalar.activation(esmT[:], z2T[:], AF.Exp, scale=-0.5, bias=0.0)

            # DVE: dvg = eLg - eHg
            dvg = sb.tile([128, 1], F32)
            nc.vector.tensor_sub(dvg[:], eLg[:], eHg[:])

            # gate chain [128, 64]: dG/selG/z2G Pool, d2G DVE
            dG = sb.tile([128, 64], F32)
            nc.gpsimd.tensor_scalar(dG[:], xitF[:], diffc[:], aLm[:],
                                    ALU.mult, ALU.add)
            selG = sb.tile([128, 64], F32)
            nc.gpsimd.tensor_scalar(selG[:], dG[:], 0.0, dvg[:],
                                    ALU.is_le, ALU.mult)
            d2G = sb.tile([128, 64], F32)
            nc.vector.tensor_mul(d2G[:], dG[:], dG[:])
            z2G = sb.tile([128, 64], F32)
            nc.vector.scalar_tensor_tensor(z2G[:], selG[:], eHg[:],
                                           d2G[:], ALU.add, ALU.mult)

            # PE: everything sigma-shaped straight from esmT; dM gets the
            # +C*eye diagonal offset accumulated in PSUM
            nc.tensor.matmul(dM[:], esmT[:], esmT[:],
                             start=True, stop=False, skip_group_check=True)
            nc.tensor.matmul(dM[:], Ceye[:], eye32[:],
                             start=False, stop=True, skip_group_check=True)
            nc.tensor.matmul(sgB[:], ones8[:], esmT[:], skip_group_check=True)
            nc.tensor.matmul(sgcol[:], esmT[:], ones8[:, 0:1],
                             skip_group_check=True)

            # DVE: accumT[l] = (K-1) * sum_k esmT[l,k]  (res-seed input)
            dumA = sb.tile([L, K], F32)
            accumT = sb.tile([L, 1], F32)
            nc.vector.scalar_tensor_tensor(dumA[:], esmT[:], float(K - 1),
                                           ones8[:], ALU.mult, ALU.mult,
                                           accum_out=accumT[:])
            # PE: res seed = (K-1)*sum(sigma) on even rows
            nc.tensor.matmul(res8[:], evens8[:], accumT[:],
                             start=True, stop=False, skip_group_check=True)

            # DVE mix block, all same-queue: dd, select, dd^2, z2M
            dd = sb.tile([K, K], F32)
            nc.vector.tensor_scalar(dd[:], dM[:], emS_c, None, ALU.subtract)
            mdM = sb.tile([K, K], F32)
            nc.vector.tensor_scalar(mdM[:], dd[:], 0.0, dvm_c,
                                    ALU.is_le, ALU.mult)
            d2M = sb.tile([K, K], F32)
            nc.vector.tensor_mul(d2M[:], dd[:], dd[:])
            z2M = sb.tile([K, K], F32)
            nc.vector.scalar_tensor_tensor(z2M[:], mdM[:], ehm_c, d2M[:],
                                           ALU.add, ALU.mult)

            # DVE: sgB to SBUF (dumU's in1; fills the Mx wait gap)
            sgBc = sb.tile([K, K], F32)
            nc.vector.tensor_scalar(sgBc[:], sgB[:], 1.0, None, ALU.mult)

            # ACT: eG then Mx (readiness order)
            eG = sb.tile([128, 64], F32)
            nc.scalar.activation(eG[:], z2G[:], AF.Exp, scale=-0.5, bias=0.0)
            Mx = sb.tile([K, K], F32)
            nc.scalar.activation(Mx[:], z2M[:], AF.Exp, scale=-0.5, bias=0.0)

            # DVE (fills a wait gap): w38 = qq8*sigma - c8
            w38 = sb.tile([K, 8], F32)
            nc.vector.scalar_tensor_tensor(w38[:], qq8, sgcol[:], c8[:],
                                           ALU.mult, ALU.subtract)

            # DVE: u_j = sum_i Mx[j,i]*sigma_i (diag masked via exp clamp)
            dumU = sb.tile([K, K], F32)
            u_c = sb.tile([K, 1], F32)
            nc.vector.scalar_tensor_tensor(dumU[:], Mx[:], 1.0, sgBc[:],
                                           ALU.mult, ALU.mult,
                                           accum_out=u_c[:])

            # PE: g8 = indic^T @ eG ; res8 += w38^T @ u
            nc.tensor.matmul(g8[:], indic, eG[:], skip_group_check=True)
            nc.tensor.matmul(res8[:], w38[:], u_c[:],
                             start=False, stop=True, skip_group_check=True)

            # DVE: out8 = g8 * res8 -> scatter source rows 0:8
            nc.vector.tensor_scalar(src[0:8, :], g8[:], res8[:], None,
                                    ALU.mult)
            nc.gpsimd.trigger_dma(count=None)

    nc.compile()
    return nc


def _prep_in_maps(inputs):
    x = np.asarray(inputs["x"], dtype=np.float64)
    sw = np.asarray(inputs["smearWindow"], dtype=np.float64)
    if not float(sw[0]) == float(sw[1]):
        raise NotImplementedError(
            "kernel specialized for smearWindow[0] == smearWindow[1] "
            "(xStep == 0); got %r" % (sw,)
        )
    low, up = float(sw[0]), float(sw[1])
    oml = 1.0 - low
    km = np.asarray(inputs["kmean"], np.float64)
    kl = np.asarray(inputs["klow"], np.float64)
    kh = np.asarray(inputs["khigh"], np.float64)
    el = np.asarray(inputs["ent_low"], np.float64)
    eh = np.asarray(inputs["ent_high"], np.float64)
    em = np.asarray(inputs["ent_mean"], np.float64)
    pol = np.asarray(inputs["pol"], np.float64)
    pre = np.asarray(inputs["pol_re"], np.float64)[:, 0, 0]
    pim = np.asarray(inputs["pol_im"], np.float64)[:, 0, 0]
    aw = np.asarray(inputs["attn_w"], np.float64)
    ab = np.asarray(inputs["attn_b"], np.float64)
    asc = np.asarray(inputs["attn_scope"], np.float64)

    base = np.zeros((128, W), dtype=np.float64)

    def put(col, vals, width=1):
        v = np.asarray(vals)
        if v.ndim == 1:
            v = v[:, None]
        for g in range(4):
            base[32 * g:32 * g + 32, col:col + width] = v

    ekl = np.exp(-2.0 * kl) * oml * oml
    ekh = np.exp(-2.0 * kh) * oml * oml
    base[0:L, C_KMT:C_KMT + K] = (km / oml).T
    base[0:L, C_DVST:C_DVST + K] = (ekl - ekh).T
    base[0:L, C_EAHT:C_EAHT + K] = ekh.T
    eel = np.exp(-2.0 * el) * (S * S)
    eeh = np.exp(-2.0 * eh) * (S * S)
    put(C_EMS, em / S)
    put(C_DVM, eel - eeh)
    put(C_EHM, eeh)
    sinp = np.sin(pol + math.pi / 4.0) * SQ2S
    qq8 = np.zeros((K, 8))
    qq8[:, 0:8:2] = (pre * sinp)[:, None]
    qq8[:, 1:8:2] = (pim * sinp)[:, None]
    put(C_QQ8, qq8, 8)
    put(C_AW, -aw)
    put(C_AB, -ab)
    t34l = 1.0 - low * asc
    t34h = 1.0 + up * asc
    put(C_T34LK, t34l / K)
    put(C_DLHK, (t34h - t34l) / K)
    put(C_M2L, -2.0 * t34l / K)
    put(C_M2H, -2.0 * t34h / K)
    for r in range(8):
        blk = r // 2
        base[32 * blk:32 * blk + 32, C_INDIC + r] = 1.0

    in_maps = []
    for b in range(B):
        cb = base.copy()
        for g in range(4):
            cb[32 * g:32 * g + 32, C_X] = x[b]
        cb[0:L, C_XB8:C_XB8 + K] = x[b][None, :]
        in_maps.append({"cols": cb.astype(np.float32)})
    return in_maps


def _unpack_out(o: np.ndarray) -> np.ndarray:
    re = np.concatenate([o[2 * q] for q in range(4)])
    im = np.concatenate([o[2 * q + 1] for q in range(4)])
    return re + 1j * im


LAST_RESULTS = None


def kernel(**inputs) -> np.ndarray:
    global LAST_RESULTS
    import os

    if "nc" not in _NC_CACHE:
        _NC_CACHE["nc"] = _build_nc()
    nc = _NC_CACHE["nc"]
    in_maps = _prep_in_maps(inputs)
    trace = bool(int(os.environ.get("KNOT_TRACE", "0")))
    r = bass_utils.run_bass_kernel_spmd(
        nc, in_maps, core_ids=list(range(B)), trace=trace
    )
    LAST_RESULTS = r
    out = np.empty((B, S), dtype=np.complex64)
    for b in range(B):
        out[b] = _unpack_out(r.results[b]["out"]).astype(np.complex64)
    return out
